# revision 20
# baseline (speedup 1.0000x reference)
"""Trainium2 Bass kernel: batched 8x8-block IDCT (dequant + 2D separable transform).

Math per 8x8 block b of each 1024x1024 image:
    out_b = mtx.T @ (qtable * b) @ mtx + 128

Implementation (per core, pure data parallel over the batch dim):
  - Each of 8 cores gets 4 images = 4096 rows x 1024 cols, processed as 32
    slabs of 128 rows.
  - Per slab: dequantize elementwise with a pre-tiled qtable (DVE), then for
    each 128x128 chunk two matmuls with the data as the *stationary* operand
    and C = kron(I_16, mtx) as the moving operand:
        P1_c = Xd_c^T @ C    (row-pass, output lands transposed: (w, i))
        P2_c = P1_c^T @ C    (col-pass, output back in (i, j) orientation)
    The +128 rides on the final PSUM->SBUF copy.
  - Host side only shards/gathers and builds the two small constants.
"""

import numpy as np

_N_CORES = 8
_B, _H, _W = 32, 1024, 1024
_PER = _B // _N_CORES            # images per core
_ROWS = _PER * _H                # 4096 rows per core
_SLABS = _ROWS // 128            # 32 slabs of 128 rows

_nc_cache = None


def _build_nc():
    from contextlib import ExitStack

    import concourse.bass as bass
    import concourse.tile as tile
    from concourse import mybir

    F32 = mybir.dt.float32
    nc = bass.Bass()
    x_in = nc.declare_dram_parameter("x", [_ROWS, _W], F32, isOutput=False)
    qt_in = nc.declare_dram_parameter("qtile", [128, _W], F32, isOutput=False)
    c_in = nc.declare_dram_parameter("cmat", [128, 128], F32, isOutput=False)
    y_out = nc.declare_dram_parameter("y", [_ROWS, _W], F32, isOutput=True)

    with ExitStack() as ctx:
        tc = ctx.enter_context(tile.TileContext(nc))
        const = ctx.enter_context(tc.tile_pool(name="const", bufs=1))
        xp = ctx.enter_context(tc.tile_pool(name="xp", bufs=3))
        xdp = ctx.enter_context(tc.tile_pool(name="xdp", bufs=3))
        s1p = ctx.enter_context(tc.tile_pool(name="s1p", bufs=4))
        op = ctx.enter_context(tc.tile_pool(name="op", bufs=3))
        p1p = ctx.enter_context(tc.tile_pool(name="p1p", bufs=2, space="PSUM"))
        p2p = ctx.enter_context(tc.tile_pool(name="p2p", bufs=2, space="PSUM"))

        qt = const.tile([128, _W], F32)
        nc.sync.dma_start(qt[:], qt_in[:])
        cm = const.tile([128, 128], F32)
        nc.scalar.dma_start(cm[:], c_in[:])

        # Touch the constants once so their DMA waits are absorbed here;
        # steady-state instructions then carry a single wait each (walrus
        # rejects instructions with too many sync waits).
        scratch = const.tile([128, 1], F32)
        nc.vector.tensor_copy(scratch[:], qt[:, :1])
        p1 = p1p.tile([128, _W], F32)
        nc.tensor.matmul(p1[:, :8], cm[:], cm[:, :8], start=True, stop=True)

        def emit_pass2(s, s1):
            p2 = p2p.tile([128, _W], F32)
            for c in range(8):
                nc.tensor.matmul(
                    p2[:, 128 * c : 128 * (c + 1)],
                    s1[:, 128 * c : 128 * (c + 1)],
                    cm[:],
                    start=True,
                    stop=True,
                )
            ot = op.tile([128, _W], F32)
            nc.vector.tensor_scalar_add(ot[:], p2[:], 128.0)
            nc.sync.dma_start(y_out[128 * s : 128 * (s + 1), :], ot[:])

        # Software-pipeline the two matmul passes by one slab: emit mm1(s)
        # before mm2(s-1) so every PE group's cross-engine dependency is a
        # full slab old and its semaphore wait is pre-satisfied.
        pending = []
        for s in range(_SLABS):
            xt = xp.tile([128, _W], F32)
            nc.sync.dma_start(xt[:], x_in[128 * s : 128 * (s + 1), :])

            xd = xdp.tile([128, _W], F32)
            nc.vector.tensor_mul(xd[:], xt[:], qt[:])

            p1 = p1p.tile([128, _W], F32)
            for c in range(8):
                nc.tensor.matmul(
                    p1[:, 128 * c : 128 * (c + 1)],
                    xd[:, 128 * c : 128 * (c + 1)],
                    cm[:],
                    start=True,
                    stop=True,
                )

            s1 = s1p.tile([128, _W], F32)
            nc.scalar.copy(s1[:], p1[:])

            pending.append((s, s1))
            if len(pending) > 2:
                emit_pass2(*pending.pop(0))

        while pending:
            emit_pass2(*pending.pop(0))

    _split_excess_waits(nc, mybir)
    return nc


def _split_excess_waits(nc, mybir):
    """Walrus allows a limited number of sync waits per lowered instruction
    (1 for DMA/DVE/ACT structs, a couple for matmul via the LDWEIGHTS pair,
    2 per EventSemaphore). Tile's wait assignment can attach more; move the
    excess onto standalone same-engine EventSemaphore carriers."""

    def budget(inst):
        tn = type(inst).__name__
        if tn == "InstEventSemaphore":
            return 2
        return 1

    wid = 0
    for fn in nc.m.functions:
        for bb in fn.blocks:
            out = []
            for inst in bb.instructions:
                si = inst.sync_info
                waits = list(si.on_wait) if si is not None else []
                b = budget(inst)
                if len(waits) > b:
                    extra, keep = waits[:-b], waits[-b:]
                    for i in range(0, len(extra), 2):
                        ev = mybir.InstEventSemaphore(
                            name=f"WSPLIT-{wid}", ins=[], outs=[]
                        )
                        wid += 1
                        ev.engine = inst.engine
                        ev.sync_info = mybir.SyncInfo(
                            on_wait=extra[i : i + 2], on_update=[]
                        )
                        out.append(ev)
                    inst.sync_info = mybir.SyncInfo(
                        on_wait=keep, on_update=list(si.on_update)
                    )
                out.append(inst)
            bb.instructions = out


def _get_nc():
    global _nc_cache
    if _nc_cache is None:
        _nc_cache = _build_nc()
    return _nc_cache


def _run(x, qtable, mtx, trace=False, **kwargs):
    from concourse.bass_utils import run_bass_kernel_spmd

    x = np.ascontiguousarray(np.asarray(x, dtype=np.float32)).reshape(_B * _H, _W)
    qtable = np.asarray(qtable, dtype=np.float32)
    mtx = np.asarray(mtx, dtype=np.float32)
    qtile = np.ascontiguousarray(np.tile(qtable, (16, _W // 8)))
    cmat = np.ascontiguousarray(np.kron(np.eye(16, dtype=np.float32), mtx))

    in_maps = [
        {
            "x": np.ascontiguousarray(x[i * _ROWS : (i + 1) * _ROWS]),
            "qtile": qtile,
            "cmat": cmat,
        }
        for i in range(_N_CORES)
    ]
    res = run_bass_kernel_spmd(
        _get_nc(), in_maps, list(range(_N_CORES)), trace=trace, **kwargs
    )
    out = np.concatenate([res.results[i]["y"] for i in range(_N_CORES)], axis=0)
    return out.reshape(_B, 1, _H, _W).astype(np.float32, copy=False), res


def kernel(x, qtable, mtx):
    out, _ = _run(x, qtable, mtx, trace=False)
    return out


# revision 21
# speedup vs baseline: 1.0062x; 1.0062x over previous
"""Trainium2 Bass kernel: batched 8x8-block IDCT (dequant + 2D separable transform).

Math per 8x8 block b of each 1024x1024 image:
    out_b = mtx.T @ (qtable * b) @ mtx + 128

Implementation (per core, pure data parallel over the batch dim):
  - Each of 8 cores gets 4 images = 4096 rows x 1024 cols, processed as 32
    slabs of 128 rows.
  - Per slab: dequantize elementwise with a pre-tiled qtable (DVE), then for
    each 128x128 chunk two matmuls with the data as the *stationary* operand
    and C = kron(I_16, mtx) as the moving operand:
        P1_c = Xd_c^T @ C    (row-pass, output lands transposed: (w, i))
        P2_c = P1_c^T @ C    (col-pass, output back in (i, j) orientation)
    The +128 rides on the final PSUM->SBUF copy.
  - Host side only shards/gathers and builds the two small constants.
"""

import numpy as np

_N_CORES = 8
_B, _H, _W = 32, 1024, 1024
_PER = _B // _N_CORES            # images per core
_ROWS = _PER * _H                # 4096 rows per core
_SLABS = _ROWS // 128            # 32 slabs of 128 rows

_nc_cache = None


def _build_nc():
    from contextlib import ExitStack

    import concourse.bass as bass
    import concourse.tile as tile
    from concourse import mybir

    F32 = mybir.dt.float32
    nc = bass.Bass()
    x_in = nc.declare_dram_parameter("x", [_ROWS, _W], F32, isOutput=False)
    qt_in = nc.declare_dram_parameter("qtile", [128, _W], F32, isOutput=False)
    c_in = nc.declare_dram_parameter("cmat", [128, 128], F32, isOutput=False)
    y_out = nc.declare_dram_parameter("y", [_ROWS, _W], F32, isOutput=True)

    with ExitStack() as ctx:
        tc = ctx.enter_context(tile.TileContext(nc))
        const = ctx.enter_context(tc.tile_pool(name="const", bufs=1))
        xp = ctx.enter_context(tc.tile_pool(name="xp", bufs=3))
        xdp = ctx.enter_context(tc.tile_pool(name="xdp", bufs=3))
        s1p = ctx.enter_context(tc.tile_pool(name="s1p", bufs=3))
        op = ctx.enter_context(tc.tile_pool(name="op", bufs=3))
        p1p = ctx.enter_context(tc.tile_pool(name="p1p", bufs=2, space="PSUM"))
        p2p = ctx.enter_context(tc.tile_pool(name="p2p", bufs=2, space="PSUM"))

        qt = const.tile([128, _W], F32)
        nc.sync.dma_start(qt[:], qt_in[:])
        cm = const.tile([128, 128], F32)
        nc.sync.dma_start(cm[:], c_in[:])

        # Touch the constants once so their DMA waits are absorbed here;
        # steady-state instructions then carry a single wait each (walrus
        # rejects instructions with too many sync waits).
        scratch = const.tile([128, 1], F32)
        nc.vector.tensor_copy(scratch[:], qt[:, :1])
        p1 = p1p.tile([128, _W], F32)
        nc.tensor.matmul(p1[:, :8], cm[:], cm[:, :8], start=True, stop=True)

        def emit_pass2(s, s1):
            p2 = p2p.tile([128, _W], F32)
            for c in range(8):
                nc.tensor.matmul(
                    p2[:, 128 * c : 128 * (c + 1)],
                    s1[:, 128 * c : 128 * (c + 1)],
                    cm[:],
                    start=True,
                    stop=True,
                )
            ot = op.tile([128, _W], F32)
            nc.vector.tensor_scalar_add(ot[:], p2[:], 128.0)
            nc.sync.dma_start(y_out[128 * s : 128 * (s + 1), :], ot[:])

        # Software-pipeline the two matmul passes by one slab: emit mm1(s)
        # before mm2(s-1) so every PE group's cross-engine dependency is a
        # full slab old and its semaphore wait is pre-satisfied.
        prev = None
        for s in range(_SLABS):
            xt = xp.tile([128, _W], F32)
            nc.sync.dma_start(xt[:], x_in[128 * s : 128 * (s + 1), :])

            xd = xdp.tile([128, _W], F32)
            nc.vector.tensor_mul(xd[:], xt[:], qt[:])

            p1 = p1p.tile([128, _W], F32)
            for c in range(8):
                nc.tensor.matmul(
                    p1[:, 128 * c : 128 * (c + 1)],
                    xd[:, 128 * c : 128 * (c + 1)],
                    cm[:],
                    start=True,
                    stop=True,
                )

            s1 = s1p.tile([128, _W], F32)
            nc.scalar.copy(s1[:], p1[:])

            if prev is not None:
                emit_pass2(*prev)
            prev = (s, s1)

        emit_pass2(*prev)

    _split_excess_waits(nc, mybir)
    return nc


def _split_excess_waits(nc, mybir):
    """Walrus allows a limited number of sync waits per lowered instruction
    (1 for DMA/DVE/ACT structs, a couple for matmul via the LDWEIGHTS pair,
    2 per EventSemaphore). Tile's wait assignment can attach more; move the
    excess onto standalone same-engine EventSemaphore carriers."""

    def budget(inst):
        tn = type(inst).__name__
        if tn == "InstEventSemaphore":
            return 2
        return 1

    wid = 0
    for fn in nc.m.functions:
        for bb in fn.blocks:
            out = []
            for inst in bb.instructions:
                si = inst.sync_info
                waits = list(si.on_wait) if si is not None else []
                b = budget(inst)
                if len(waits) > b:
                    extra, keep = waits[:-b], waits[-b:]
                    for i in range(0, len(extra), 2):
                        ev = mybir.InstEventSemaphore(
                            name=f"WSPLIT-{wid}", ins=[], outs=[]
                        )
                        wid += 1
                        ev.engine = inst.engine
                        ev.sync_info = mybir.SyncInfo(
                            on_wait=extra[i : i + 2], on_update=[]
                        )
                        out.append(ev)
                    inst.sync_info = mybir.SyncInfo(
                        on_wait=keep, on_update=list(si.on_update)
                    )
                out.append(inst)
            bb.instructions = out


def _get_nc():
    global _nc_cache
    if _nc_cache is None:
        _nc_cache = _build_nc()
    return _nc_cache


def _run(x, qtable, mtx, trace=False, **kwargs):
    from concourse.bass_utils import run_bass_kernel_spmd

    x = np.ascontiguousarray(np.asarray(x, dtype=np.float32)).reshape(_B * _H, _W)
    qtable = np.asarray(qtable, dtype=np.float32)
    mtx = np.asarray(mtx, dtype=np.float32)
    qtile = np.ascontiguousarray(np.tile(qtable, (16, _W // 8)))
    cmat = np.ascontiguousarray(np.kron(np.eye(16, dtype=np.float32), mtx))

    in_maps = [
        {
            "x": np.ascontiguousarray(x[i * _ROWS : (i + 1) * _ROWS]),
            "qtile": qtile,
            "cmat": cmat,
        }
        for i in range(_N_CORES)
    ]
    res = run_bass_kernel_spmd(
        _get_nc(), in_maps, list(range(_N_CORES)), trace=trace, **kwargs
    )
    out = np.concatenate([res.results[i]["y"] for i in range(_N_CORES)], axis=0)
    return out.reshape(_B, 1, _H, _W).astype(np.float32, copy=False), res


def kernel(x, qtable, mtx):
    out, _ = _run(x, qtable, mtx, trace=False)
    return out
